# revision 1
# baseline (speedup 1.0000x reference)
"""Trainium2 Bass kernel for nn_MoEConnectionProcessor (v2: all-blockT).

Strategy
--------
Data-parallel over 8 cores (32768 cells each). Per core, cells are processed
in super-tiles (ST) of 2048 cells laid out "blockT": SBUF partition =
(g, d) with g = cell-subgroup (4 of 32 cells within a 128-cell tile),
d = feature; free axis = (t, c) = (tile-in-ST, cell-in-subgroup) = 512 cols.

The host pre-sorts each cell's 26 neighbors by tier and stages THREE
premasked copies of neighbor_states (tier-0/1/2 * nb), truncated to the
global max per-tier count W_t (~21), already in blockT with j (neighbor
slot) outermost. Because the masks are 0/1 and the tier classes partition
the neighbors:

  - S_t = sum_k m_t*nb   becomes an UNMASKED PE accumulation chain over j
    (identity stationary, premasked zeros contribute nothing) -> no DVE
    mask products, no transposes, no reduces.
  - tanh(m1 * msg) = m1 * tanh(msg) (b_msg == 0 per spec), so the
    functional expert's masked message sum is: matmul kron(I4, W_msg) per
    j-slot -> ACT tanh -> PE accumulation chain.
  - S0 = S_t0 + S_t1 + S_t2 (two cheap adds), loc_mean = S_t0/cnt0, etc.

All experts, gating, CNF steps and the final combine run in blockT
(biases are per-partition there). Gates ([12, 512] = (g, expert) rows)
are broadcast to 128 partitions with tiny scatter matmuls. Output stays
blockT in DRAM; the host inverse-permutes.

sigmoid(x) = 0.5*tanh(0.5x) + 0.5 and relu on DVE keep every ACT function
in one activation-table set (no ACT_TABLE_LOAD churn).
"""

import numpy as np
import ml_dtypes
from contextlib import ExitStack

import concourse.bass as bass
import concourse.bacc as bacc
import concourse.tile as tile
import concourse.mybir as mybir

B, K, D, NH = 262144, 26, 32, 32
N_CORES = 8
BS = B // N_CORES          # 32768 cells per core
ST = 2048                  # cells per super-tile
NT = BS // ST              # 16 super-tiles per core
TPS = ST // 128            # 16 tiles of 128 cells per super-tile
SC = TPS * 32              # 512 free columns per super-tile (t, c)
N_STEPS = 3
DT_STEP = 1.0 / N_STEPS

dt = mybir.dt
bf16 = ml_dtypes.bfloat16
f8e4 = ml_dtypes.float8_e4m3
AF = mybir.ActivationFunctionType
ALU = mybir.AluOpType

# staged dtype of the three big premasked neighbor copies
STAGE_DT = dt.bfloat16
STAGE_NP = bf16

# stationary slots in the packed weight tensor [128, n*128 + 12 + 4 + 3*128 + 12]
_WSLOTS = ["I128", "W4msg", "Wl_t", "Wl_b", "Wu_t", "Wu_b", "Wc_t", "Wc_b",
           "Wg1_t", "Wg1_b", "Wc_td"]
# extra (non-128-wide) stationaries appended after the slots:
#   kron(I4, W_g2):      [128, 12]
#   ones_sum:            [12, 4]   (pad part-dim to 12 rows used)
#   recip bcast (f32):   [4, 12]
#   gate scatter e=0..2: [12, 128] each
EX_G2 = 128 * len(_WSLOTS)
EX_ONES = EX_G2 + 12
EX_SCAT = EX_ONES + 4          # 3x128 bf16 scatter
WC_COLS = EX_SCAT + 3 * 128
WF_COLS = 140                  # f32: recip bcast [4, 12] + gate-div bcast [4, 128]
BC_COLS = 8                    # f32 biases


def _wslot(name):
    return 128 * _WSLOTS.index(name)


def build_program(w0, w1, w2):
    nc = bacc.Bacc("TRN2", target_bir_lowering=False, debug=False,
                   num_devices=N_CORES)

    a_m0 = nc.dram_tensor("m0", [128, sum(w0) * SC], dt.bfloat16, kind="ExternalInput").ap()
    a_m1 = nc.dram_tensor("m1", [128, sum(w1) * SC], dt.bfloat16, kind="ExternalInput").ap()
    a_m2 = nc.dram_tensor("m2", [128, sum(w2) * SC], dt.bfloat16, kind="ExternalInput").ap()
    a_cst = nc.dram_tensor("cst", [128, NT * SC], dt.bfloat16, kind="ExternalInput").ap()
    a_icn = nc.dram_tensor("icn", [128, NT * 3 * SC], dt.bfloat16, kind="ExternalInput").ap()
    a_wc = nc.dram_tensor("wc", [128, WC_COLS], dt.bfloat16, kind="ExternalInput").ap()
    a_wf = nc.dram_tensor("wf", [4, WF_COLS], dt.float32, kind="ExternalInput").ap()
    a_bc = nc.dram_tensor("bc", [128, BC_COLS], dt.float32, kind="ExternalInput").ap()
    a_out = nc.dram_tensor("out", [128, NT * SC], dt.bfloat16, kind="ExternalOutput").ap()

    with tile.TileContext(nc) as tc:
        _body(tc, a_m0, a_m1, a_m2, a_cst, a_icn, a_wc, a_wf, a_bc,
              a_out, w0, w1, w2)
    nc.compile()
    return nc


def _body(tc, a_m0, a_m1, a_m2, a_cst, a_icn, a_wc, a_wf, a_bc,
          a_out, w0, w1, w2):
    nc = tc.nc

    with ExitStack() as ctx:
        cpool = ctx.enter_context(tc.tile_pool(name="const", bufs=1))
        pin0 = ctx.enter_context(tc.tile_pool(name="in0", bufs=2))
        pin1 = ctx.enter_context(tc.tile_pool(name="in1", bufs=2))
        pin2 = ctx.enter_context(tc.tile_pool(name="in2", bufs=2))
        pinc = ctx.enter_context(tc.tile_pool(name="inc", bufs=2))
        ptnh = ctx.enter_context(tc.tile_pool(name="tnh", bufs=2))
        psml = ctx.enter_context(tc.tile_pool(name="sml", bufs=1))
        pout = ctx.enter_context(tc.tile_pool(name="out", bufs=2))
        # PSUM: chains (St1|St2) 2 banks x2 + msgs 2 + experts 2 = 8
        pps_ch = ctx.enter_context(tc.tile_pool(name="pch", bufs=2, space="PSUM"))
        pps_m = ctx.enter_context(tc.tile_pool(name="pm", bufs=2, space="PSUM"))
        pps_e = ctx.enter_context(tc.tile_pool(name="pe", bufs=2, space="PSUM"))

        wc = cpool.tile([128, WC_COLS], dt.bfloat16, tag="wc")
        nc.sync.dma_start(wc[:], a_wc)
        wf = cpool.tile([4, WF_COLS], dt.float32, tag="wf")
        nc.sync.dma_start(wf[:], a_wf)
        bc = cpool.tile([128, BC_COLS], dt.float32, tag="bc")
        nc.sync.dma_start(bc[:], a_bc)

        def W(name):
            return wc[:, _wslot(name): _wslot(name) + 128]

        kron_g2 = wc[:, EX_G2:EX_G2 + 12]
        ones_sum = wc[0:12, EX_ONES:EX_ONES + 4]
        rb_f32 = wf[0:4, 0:12]
        div_f32 = wf[0:4, 12:140]
        scat = [wc[0:12, EX_SCAT + 128 * e: EX_SCAT + 128 * (e + 1)]
                for e in range(3)]

        b_loc4 = bc[:, 1:2]
        b_updh = bc[:, 2:3]   # 0.5 * b_upd, for sigmoid-via-tanh
        b_cnf4 = bc[:, 3:4]
        b_g14 = bc[:, 4:5]
        b_g2r = bc[0:12, 5:6]  # b_g2 on (g,e) rows 0..11
        b_msg4 = bc[:, 0:1]

        offs = [0, 0, 0]

        def emit_loads(i):
            v0, v1, v2 = w0[i], w1[i], w2[i]
            st = {}
            m0 = pin0.tile([128, v0 * SC], dt.bfloat16, tag="m0")
            nc.sync.dma_start(m0[:], a_m0[:, offs[0] * SC:(offs[0] + v0) * SC])
            m1 = pin1.tile([128, v1 * SC], dt.bfloat16, tag="m1")
            nc.sync.dma_start(m1[:], a_m1[:, offs[1] * SC:(offs[1] + v1) * SC])
            m2 = pin2.tile([128, v2 * SC], dt.bfloat16, tag="m2")
            nc.sync.dma_start(m2[:], a_m2[:, offs[2] * SC:(offs[2] + v2) * SC])
            cst = pinc.tile([128, SC], dt.bfloat16, tag="cst")
            nc.sync.dma_start(cst[:], a_cst[:, i * SC:(i + 1) * SC])
            icn = pinc.tile([128, 3 * SC], dt.bfloat16, tag="icn")
            nc.sync.dma_start(icn[:], a_icn[:, i * 3 * SC:(i + 1) * 3 * SC])
            offs[0] += v0; offs[1] += v1; offs[2] += v2
            tnh = ptnh.tile([128, max(w1) * SC], dt.bfloat16, tag="tnh")
            pch = pps_ch.tile([128, 2 * SC], dt.float32, tag="ch")
            st.update(m0=m0, m1=m1, m2=m2, cst=cst, icn=icn, tnh=tnh,
                      pch=pch, v0=v0, v1=v1, v2=v2)
            return st

        def chain_steps(st):
            """Thunks: St1/St2/msgs chains round-robin, then m0 fold levels."""
            m0, m1, m2, tnh, pch = st["m0"], st["m1"], st["m2"], st["tnh"], st["pch"]
            v0, v1, v2 = st["v0"], st["v1"], st["v2"]
            pSt1 = pch[:, 0:SC]
            pSt2 = pch[:, SC:2 * SC]
            st["pSt1"], st["pSt2"] = pSt1, pSt2
            steps = []

            def mk_msg(j):
                def f():
                    pm = pps_m.tile([128, SC], dt.float32, tag="pm")
                    nc.tensor.matmul(pm[:], W("W4msg"), m1[:, j * SC:(j + 1) * SC],
                                     start=True, stop=True)
                    nc.scalar.activation(tnh[:, j * SC:(j + 1) * SC], pm[:],
                                         AF.Tanh, bias=b_msg4, scale=1.0)
                return f

            def mk_s1(j):
                return lambda: nc.tensor.matmul(
                    pSt1, W("I128"), m1[:, j * SC:(j + 1) * SC],
                    start=(j == 0), stop=(j == v1 - 1))

            def mk_s2(j):
                return lambda: nc.tensor.matmul(
                    pSt2, W("I128"), m2[:, j * SC:(j + 1) * SC],
                    start=(j == 0), stop=(j == v2 - 1))

            for j in range(max(v1, v2)):
                if j < v1:
                    steps.append(mk_msg(j))
                    steps.append(mk_s1(j))
                if j < v2:
                    steps.append(mk_s2(j))

            def mk_fold(h, fw, first):
                def f():
                    eng = nc.gpsimd if first else nc.vector
                    eng.tensor_tensor(
                        out=m0[:, 0:h * SC], in0=m0[:, 0:h * SC],
                        in1=m0[:, (fw - h) * SC:fw * SC], op=ALU.add)
                return f

            fw = v0
            first = True
            while fw > 1:
                h = fw // 2
                steps.append(mk_fold(h, fw, first))
                fw = fw - h
                first = False
            return steps

        def back_units(i, st):
            """Thunks: means, experts, gating, CNF, combine for super-tile i."""
            m0, cst, icn = st["m0"], st["cst"], st["icn"]
            pSt1, pSt2, tnh, v1 = st["pSt1"], st["pSt2"], st["tnh"], st["v1"]
            inv0 = icn[:, 0:SC]
            inv1 = icn[:, SC:2 * SC]
            inv2 = icn[:, 2 * SC:3 * SC]
            units = []
            h = {}

            def u_aggfold():
                fw = v1
                while fw > 1:
                    hh = fw // 2
                    nc.vector.tensor_tensor(
                        out=tnh[:, 0:hh * SC], in0=tnh[:, 0:hh * SC],
                        in1=tnh[:, (fw - hh) * SC:fw * SC], op=ALU.add)
                    fw = fw - hh
            units.append(u_aggfold)

            def u_means():
                h["mloc"] = psml.tile([128, SC], dt.bfloat16, name="mloc", tag="mloc")
                nc.vector.tensor_tensor(out=h["mloc"][:], in0=m0[:, 0:SC], in1=inv0, op=ALU.mult)
                h["mdis"] = psml.tile([128, SC], dt.bfloat16, name="mdis", tag="mdis")
                nc.vector.tensor_tensor(out=h["mdis"][:], in0=pSt2, in1=inv2, op=ALU.mult)
            units.append(u_means)

            def u_s0():
                h["st1c"] = psml.tile([128, SC], dt.bfloat16, name="st1c", tag="st1c")
                nc.scalar.copy(h["st1c"][:], pSt1)
                h["s01"] = psml.tile([128, SC], dt.bfloat16, name="s01", tag="s01")
                nc.vector.tensor_tensor(out=h["s01"][:], in0=m0[:, 0:SC], in1=h["st1c"][:], op=ALU.add)
                h["s0"] = psml.tile([128, SC], dt.bfloat16, name="s0", tag="s0")
                nc.vector.tensor_tensor(out=h["s0"][:], in0=pSt2, in1=h["s01"][:], op=ALU.add)
            units.append(u_s0)

            def u_aggacc():
                h["aggb"] = psml.tile([128, SC], dt.bfloat16, name="aggb", tag="aggb")
                nc.vector.tensor_tensor(out=h["aggb"][:], in0=tnh[:, 0:SC], in1=inv1, op=ALU.mult)
            units.append(u_aggacc)

            def u_local():
                pl = pps_e.tile([128, SC], dt.float32, tag="pe")
                nc.tensor.matmul(pl[:], W("Wl_t"), cst[:], start=True, stop=False)
                nc.tensor.matmul(pl[:], W("Wl_b"), h["mloc"][:], start=False, stop=True)
                h["locb"] = psml.tile([128, SC], dt.bfloat16, name="locb", tag="locb")
                nc.scalar.activation(h["locb"][:], pl[:], AF.Tanh, bias=b_loc4, scale=1.0)
            units.append(u_local)

            def u_func1():
                pu = pps_e.tile([128, SC], dt.float32, tag="pe")
                nc.tensor.matmul(pu[:], W("Wu_t"), cst[:], start=True, stop=False)
                nc.tensor.matmul(pu[:], W("Wu_b"), h["aggb"][:], start=False, stop=True)
                h["tu"] = psml.tile([128, SC], dt.bfloat16, name="tu", tag="tu")
                nc.scalar.activation(h["tu"][:], pu[:], AF.Tanh, bias=b_updh, scale=0.5)
                h["tagg"] = psml.tile([128, SC], dt.bfloat16, name="tagg", tag="tagg")
                nc.scalar.activation(h["tagg"][:], h["aggb"][:], AF.Tanh)
            units.append(u_func1)

            def u_func2():
                d2 = psml.tile([128, SC], dt.bfloat16, tag="d2")
                nc.vector.tensor_tensor(out=d2[:], in0=h["tagg"][:], in1=cst[:], op=ALU.subtract)
                e1 = psml.tile([128, SC], dt.bfloat16, tag="e1")
                nc.vector.scalar_tensor_tensor(out=e1[:], in0=h["tu"][:], scalar=0.5,
                                               in1=d2[:], op0=ALU.mult, op1=ALU.mult)
                e2 = psml.tile([128, SC], dt.bfloat16, tag="e2")
                nc.vector.scalar_tensor_tensor(out=e2[:], in0=d2[:], scalar=0.5,
                                               in1=cst[:], op0=ALU.mult, op1=ALU.add)
                h["funcb"] = psml.tile([128, SC], dt.bfloat16, name="funcb", tag="funcb")
                nc.vector.tensor_tensor(out=h["funcb"][:], in0=e1[:], in1=e2[:], op=ALU.add)
            units.append(u_func2)

            def u_gate1():
                pg = pps_e.tile([128, SC], dt.float32, tag="pe")
                nc.tensor.matmul(pg[:], W("Wg1_t"), cst[:], start=True, stop=False)
                nc.tensor.matmul(pg[:], W("Wg1_b"), h["s0"][:], start=False, stop=True)
                h["hb"] = psml.tile([128, SC], dt.bfloat16, name="hb", tag="hb")
                nc.vector.tensor_scalar(out=h["hb"][:], in0=pg[:], scalar1=b_g14,
                                        scalar2=0.0, op0=ALU.add, op1=ALU.max)
            units.append(u_gate1)

            def u_gate2():
                pl2 = pps_e.tile([128, SC], dt.float32, tag="pe")
                nc.tensor.matmul(pl2[0:12, :], kron_g2, h["hb"][:], start=True, stop=True)
                h["eg"] = psml.tile([12, SC], dt.bfloat16, name="eg", tag="eg")
                nc.scalar.activation(h["eg"][:], pl2[0:12, :], AF.Exp, bias=b_g2r, scale=1.0)
                ps = pps_e.tile([128, SC], dt.float32, tag="pe")
                nc.tensor.matmul(ps[0:4, :], ones_sum, h["eg"][:], start=True, stop=True)
                h["rec"] = psml.tile([4, SC], dt.float32, name="rec", tag="rec")
                nc.vector.reciprocal_approx_fast(out=h["rec"][:], in_=ps[0:4, :])
            units.append(u_gate2)

            def u_cnf0():
                pcnf = pps_e.tile([128, SC], dt.float32, tag="pe")
                nc.tensor.matmul(pcnf[:], W("Wc_t"), cst[:], start=True, stop=False)
                nc.tensor.matmul(pcnf[:], W("Wc_b"), h["mdis"][:], start=False, stop=True)
                h["pcnf"] = pcnf
                h["t0"] = psml.tile([128, SC], dt.bfloat16, name="t0", tag="t0")
                nc.scalar.activation(h["t0"][:], pcnf[:], AF.Tanh, bias=b_cnf4, scale=1.0)
                nc.tensor.matmul(pcnf[:], W("Wc_td"), h["t0"][:], start=False,
                                 stop=True, skip_group_check=True)
            units.append(u_cnf0)

            def u_cnf1():
                pcnf = h["pcnf"]
                h["t1"] = psml.tile([128, SC], dt.bfloat16, name="t1", tag="t1")
                nc.scalar.activation(h["t1"][:], pcnf[:], AF.Tanh, bias=b_cnf4, scale=1.0)
                nc.tensor.matmul(pcnf[:], W("Wc_td"), h["t1"][:], start=False,
                                 stop=True, skip_group_check=True)
                h["t01"] = psml.tile([128, SC], dt.bfloat16, name="t01", tag="t01")
                nc.vector.tensor_tensor(out=h["t01"][:], in0=h["t0"][:], in1=h["t1"][:], op=ALU.add)
            units.append(u_cnf1)

            def u_cnf2():
                pcnf = h["pcnf"]
                t2 = psml.tile([128, SC], dt.bfloat16, tag="t2")
                nc.scalar.activation(t2[:], pcnf[:], AF.Tanh, bias=b_cnf4, scale=1.0)
                tall = psml.tile([128, SC], dt.bfloat16, tag="tall")
                nc.vector.tensor_tensor(out=tall[:], in0=h["t01"][:], in1=t2[:], op=ALU.add)
                h["dist"] = psml.tile([128, SC], dt.bfloat16, name="dist", tag="dist")
                nc.vector.scalar_tensor_tensor(out=h["dist"][:], in0=tall[:], scalar=DT_STEP,
                                               in1=cst[:], op0=ALU.mult, op1=ALU.add)
            units.append(u_cnf2)

            def u_comb1():
                h["recb"] = pps_e.tile([128, SC], dt.float32, name="pe", tag="pe")
                nc.tensor.matmul(h["recb"][:], div_f32, h["rec"][:], start=True, stop=True)
                exps = [h["locb"], h["funcb"], h["dist"]]
                h["accs"] = []
                for e in range(3):
                    pge = pps_m.tile([128, SC], dt.float32, tag="pm")
                    nc.tensor.matmul(pge[:], scat[e], h["eg"][:], start=True, stop=True)
                    ae = psml.tile([128, SC], dt.bfloat16, tag=f"ae{e}")
                    nc.vector.tensor_tensor(out=ae[:], in0=pge[:], in1=exps[e][:], op=ALU.mult)
                    h["accs"].append(ae)
            units.append(u_comb1)

            def u_comb2():
                a1, a2, a3 = h["accs"]
                u12 = psml.tile([128, SC], dt.bfloat16, tag="u12")
                nc.gpsimd.tensor_tensor(out=u12[:], in0=a1[:], in1=a2[:], op=ALU.add)
                u123 = psml.tile([128, SC], dt.bfloat16, tag="u123")
                nc.vector.tensor_tensor(out=u123[:], in0=u12[:], in1=a3[:], op=ALU.add)
                outb = pout.tile([128, SC], dt.bfloat16, tag="outb")
                nc.vector.tensor_tensor(out=outb[:], in0=h["recb"][:], in1=u123[:], op=ALU.mult)
                nc.sync.dma_start(a_out[:, i * SC:(i + 1) * SC], outb[:])
            units.append(u_comb2)

            return units

        # software pipeline with fine-grained weave: back(i-1) units are
        # emitted between front(i) chain steps
        prev = None
        for i in range(NT):
            st = emit_loads(i)
            steps = chain_steps(st)
            units = back_units(i - 1, prev) if prev is not None else []
            k = 0
            n_s, n_u = len(steps), len(units)
            for n, s in enumerate(steps):
                s()
                while k < n_u and (k + 1) * n_s <= (n + 1) * n_u:
                    units[k]()
                    k += 1
            while k < n_u:
                units[k]()
                k += 1
            prev = st
        for u in back_units(NT - 1, prev):
            u()


# ---------------------------------------------------------------------------
# host staging
# ---------------------------------------------------------------------------

def _to_blockT(arr_bsd):
    """[n, d] (d == 32, n multiple of 128) -> blockT [128, n//128*32]:
    partition = g*32+d, cols = (t, c)."""
    n, d = arr_bsd.shape
    a = arr_bsd.reshape(n // 128, 4, 32, d)          # [t, g, c, d]
    a = a.transpose(1, 3, 0, 2)                      # [g, d, t, c]
    return np.ascontiguousarray(a.reshape(128, n // 4))


def _nb_blockT(nb_sel):
    """[n, w, 32] premasked sorted neighbors -> [128, w*n//4]:
    partition = g*32+d, cols = (j, t, c)."""
    n, w, d = nb_sel.shape
    a = nb_sel.reshape(n // 128, 4, 32, w, d)        # [t, g, c, j, d]
    a = a.transpose(1, 4, 3, 0, 2)                   # [g, d, j, t, c]
    return np.ascontiguousarray(a.reshape(128, w * n // 4))


def _from_blockT(arr):
    """inverse of _to_blockT per ST block: [128, NT*SC] -> [BS, 32]."""
    a = arr.reshape(4, 32, NT * TPS, 32)             # [g, d, t, c]
    a = a.transpose(2, 0, 3, 1)                      # [t, g, c, d]
    return np.ascontiguousarray(a.reshape(BS, 32))


def stage_weights(inputs):
    f32 = np.float32
    W_local = np.asarray(inputs["W_local"], f32)
    W_msg = np.asarray(inputs["W_msg"], f32)
    W_upd = np.asarray(inputs["W_upd"], f32)
    W_cnf = np.asarray(inputs["W_cnf"], f32)
    W_g1 = np.asarray(inputs["W_g1"], f32)
    W_g2 = np.asarray(inputs["W_g2"], f32)

    eye4 = np.eye(4, dtype=f32)

    def kron4(w):
        return np.kron(eye4, w)

    wparts = {
        "I128": np.eye(128, dtype=f32),
        "W4msg": kron4(W_msg),
        "Wl_t": kron4(W_local[:D]), "Wl_b": kron4(W_local[D:]),
        "Wu_t": kron4(W_upd[:D]), "Wu_b": kron4(W_upd[D:]),
        "Wc_t": kron4(W_cnf[:D]), "Wc_b": kron4(W_cnf[D:]),
        "Wg1_t": kron4(W_g1[:D]), "Wg1_b": kron4(W_g1[D:] / K),
        "Wc_td": kron4(W_cnf[:D] * (1.0 / N_STEPS)),
    }
    wc = np.zeros((128, WC_COLS), f32)
    for name in _WSLOTS:
        wc[:, _wslot(name):_wslot(name) + 128] = wparts[name]
    for g in range(4):
        wc[32 * g:32 * (g + 1), EX_G2 + 3 * g:EX_G2 + 3 * (g + 1)] = W_g2
    for g in range(4):
        for e in range(3):
            wc[3 * g + e, EX_ONES + g] = 1.0
    for e in range(3):
        for g in range(4):
            wc[3 * g + e, EX_SCAT + 128 * e + 32 * g:
               EX_SCAT + 128 * e + 32 * (g + 1)] = 1.0
    wc = wc.astype(bf16)

    wf = np.zeros((4, WF_COLS), f32)
    for g in range(4):
        wf[g, 3 * g:3 * (g + 1)] = 1.0          # recip bcast [4, 12]
        wf[g, 12 + 32 * g:12 + 32 * (g + 1)] = 1.0  # gate-div bcast [4, 128]

    bcq = np.zeros((128, BC_COLS), f32)
    bcq[:, 0] = np.tile(np.asarray(inputs["b_msg"], f32), 4)
    bcq[:, 1] = np.tile(np.asarray(inputs["b_local"], f32), 4)
    bcq[:, 2] = 0.5 * np.tile(np.asarray(inputs["b_upd"], f32), 4)
    bcq[:, 3] = np.tile(np.asarray(inputs["b_cnf"], f32), 4)
    bcq[:, 4] = np.tile(np.asarray(inputs["b_g1"], f32), 4)
    b_g2 = np.asarray(inputs["b_g2"], f32)
    for g in range(4):
        bcq[3 * g:3 * (g + 1), 5] = b_g2
    return wc, wf, bcq


def stage_inputs(inputs):
    """Returns (in_maps, widths, cell_order). Cells are globally sorted by
    tier-1 count and interleaved across cores so each super-tile band has a
    tight max tier count -> short accumulation chains."""
    f32 = np.float32
    cs = np.asarray(inputs["current_state"], f32)
    nb = np.asarray(inputs["neighbor_states"], f32)
    tiers = np.asarray(inputs["tier_ids"], np.int32)

    if np.any(np.asarray(inputs["b_msg"], f32) != 0.0):
        raise NotImplementedError("premask trick requires b_msg == 0")

    cnt = np.stack([(tiers == t).sum(-1) for t in range(3)], axis=1)  # [B,3] int
    snake_c0 = np.where(cnt[:, 1] % 2 == 0, cnt[:, 0], -cnt[:, 0])
    perm = np.lexsort((snake_c0, cnt[:, 1]))         # rank -> cell

    # per-ST band widths (shared by all cores), min 1
    widths = []
    for t in range(3):
        ct = cnt[perm, t].reshape(NT, ST * N_CORES)
        widths.append(tuple(int(max(1, m)) for m in ct.max(axis=1)))
    widths = tuple(widths)

    # per-tier neighbor sort order (global, cell-major)
    inv = (1.0 / np.maximum(cnt, 1.0)).astype(f32)   # [B, 3]
    wc, wf, bcq = stage_weights(inputs)

    wmax = [max(w) for w in widths]
    orders = []
    for t in range(3):
        order = np.argsort(tiers != t, axis=1, kind="stable")[:, :wmax[t]]
        orders.append(order)

    in_maps = []
    for c in range(N_CORES):
        cells = perm[c::N_CORES]                     # [BS] cell ids, c1-sorted
        mts = [[], [], []]
        for i in range(NT):
            cell_i = cells[i * ST:(i + 1) * ST]
            for t in range(3):
                w = widths[t][i]
                od = orders[t][cell_i, :w]
                sel = np.take_along_axis(nb[cell_i], od[:, :, None], axis=1)
                msk = np.take_along_axis(tiers[cell_i], od, axis=1) == t
                mts[t].append(_nb_blockT((sel * msk[:, :, None]).astype(f32)))
        icn = np.empty((128, NT * 3 * SC), f32)
        iv = [_to_blockT(np.repeat(inv[cells, t:t + 1], D, axis=1)) for t in range(3)]
        for i in range(NT):
            for t in range(3):
                icn[:, (3 * i + t) * SC:(3 * i + t + 1) * SC] = \
                    iv[t][:, i * SC:(i + 1) * SC]
        in_maps.append({
            "m0": np.concatenate(mts[0], axis=1).astype(bf16),
            "m1": np.concatenate(mts[1], axis=1).astype(bf16),
            "m2": np.concatenate(mts[2], axis=1).astype(bf16),
            "cst": _to_blockT(cs[cells]).astype(bf16),
            "icn": icn.astype(bf16),
            "wc": wc, "wf": wf, "bc": bcq,
        })
    return in_maps, widths, perm


_PROGRAM_CACHE = {}


def kernel(**inputs):
    from concourse.bass_utils import run_bass_kernel_spmd

    in_maps, widths, perm = stage_inputs(inputs)
    if widths not in _PROGRAM_CACHE:
        _PROGRAM_CACHE[widths] = build_program(*widths)
    nc = _PROGRAM_CACHE[widths]

    res = run_bass_kernel_spmd(nc, in_maps, core_ids=list(range(N_CORES)))
    out = np.empty((B, D), np.float32)
    for c in range(N_CORES):
        out[perm[c::N_CORES]] = _from_blockT(
            np.asarray(res.results[c]["out"], np.float32))
    return out



# revision 7
# speedup vs baseline: 1.2649x; 1.2649x over previous
"""Trainium2 Bass kernel for nn_MoEConnectionProcessor (v3: fp8 DoubleRow chains).

Strategy (delta over v2)
------------------------
Data-parallel over 8 cores; per core 16 super-tiles (ST) of 2048 cells in
blockT layout: SBUF partition = (g, d) (cell-subgroup x feature), free
axis = (t, c) = 512 cols per ST.

v3 structural changes:
  * The three premasked neighbor copies are staged as ONE concatenated
    fp8e4m3 tensor [tier0 | tier1 | tier2] with per-band widths padded to
    EVEN.  m0/m2 use host-side error-feedback quantization (per-cell carry
    across slots) so the tier SUMS keep near-bf16 accuracy.
  * All slot sums run on the PE as DoubleRow fp8 accumulation chains
    (2 slots per matmul) with a single shared stationary (I128 pair):
      bank B: tier2 -> (read S2) -> continue tier1 -> S12
      bank A: tier0 -> S0
      bank C: tanh(msg) slots (fp8 pairs written by ACT) -> T1
    s0 = S0 + S12, mdis = S2*inv2, mloc = S0*inv0, agg = T1*inv1.
    This removes all DVE fold trees.
  * msg matmuls: bf16 kron4(W_msg) stationary x fp8 slots (mixed dtypes),
    two slots into a 2-bank PSUM tile, ONE wide ACT tanh per pair writing
    fp8 pairs for the DR tanh-sum chain.
  * All biases are zero by spec -> dropped (asserted at staging); the
    sigmoid 1/2 scale is folded into W_upd so local/upd share plain tanh.
  * Gates are normalized (exp * 1/sum) before the scatter broadcast,
    removing the reciprocal-broadcast matmul and final divide.
  * PE weight churn minimized: chains+tanh-sum share one DR stationary.

PSUM banks: A(1) B(1) C(1) + msg pairs 2x[128,2SC](4) + expert(1) = 8.
"""

import numpy as np
import ml_dtypes
from contextlib import ExitStack

import concourse.bass as bass
import concourse.bacc as bacc
import concourse.tile as tile
import concourse.mybir as mybir

B, K, D, NH = 262144, 26, 32, 32
N_CORES = 8
BS = B // N_CORES          # 32768 cells per core
ST = 2048                  # cells per super-tile
NT = BS // ST              # 16 super-tiles per core
TPS = ST // 128            # 16 tiles of 128 cells per super-tile
SC = TPS * 32              # 512 free columns per super-tile (t, c)
N_STEPS = 3
DT_STEP = 1.0 / N_STEPS

dt = mybir.dt
bf16 = ml_dtypes.bfloat16
f8e4 = ml_dtypes.float8_e4m3
AF = mybir.ActivationFunctionType
ALU = mybir.AluOpType
PM = mybir.MatmulPerfMode

# bf16 stationary slots in wc: [128, n*128 + 12 + 12 + 3*128]
_WSLOTS = ["W4msg", "Wl_t", "Wl_b", "Wu_t", "Wu_b", "Wc_t", "Wc_b",
           "Wg1_t", "Wg1_b", "Wc_td"]
EX_G2 = 128 * len(_WSLOTS)          # kron(I4, W_g2): [128, 12]
EX_ONES = EX_G2 + 12                # ones12: [12, 12] group-sum bcast
EX_SCAT = EX_ONES + 12              # gate scatter e=0..2: [12, 128] each
WC_COLS = EX_SCAT + 3 * 128


def _wslot(name):
    return 128 * _WSLOTS.index(name)


def build_program(widths):
    nc = bacc.Bacc("TRN2", target_bir_lowering=False, debug=False,
                   num_devices=N_CORES)

    totc = sum(v0 + v1 + v2 for v0, v1, v2 in zip(*widths))
    a_ma = nc.dram_tensor("ma", [128, totc * SC], dt.float8e4,
                          kind="ExternalInput").ap()
    a_ci = nc.dram_tensor("ci", [128, NT * 4 * SC], dt.bfloat16,
                          kind="ExternalInput").ap()
    a_wc = nc.dram_tensor("wc", [128, WC_COLS], dt.bfloat16,
                          kind="ExternalInput").ap()
    a_wdr = nc.dram_tensor("wdr", [128, 2 * 128], dt.float8e4,
                           kind="ExternalInput").ap()
    a_out = nc.dram_tensor("out", [128, NT * SC], dt.bfloat16,
                           kind="ExternalOutput").ap()

    with tile.TileContext(nc) as tc:
        _body(tc, a_ma, a_ci, a_wc, a_wdr, a_out, widths)
    nc.compile()
    return nc


def _body(tc, a_ma, a_ci, a_wc, a_wdr, a_out, widths):
    nc = tc.nc
    w0s, w1s, w2s = widths

    with ExitStack() as ctx:
        cpool = ctx.enter_context(tc.tile_pool(name="const", bufs=1))
        pma = ctx.enter_context(tc.tile_pool(name="ma", bufs=2))
        pci = ctx.enter_context(tc.tile_pool(name="ci", bufs=2))
        ptnh = ctx.enter_context(tc.tile_pool(name="tnh", bufs=2))
        psml = ctx.enter_context(tc.tile_pool(name="sml", bufs=1))
        pout = ctx.enter_context(tc.tile_pool(name="out", bufs=2))
        # PSUM: A(1) + B(1) + C(1) + msg 2x2 + expert 1 = 8 banks
        ppA = ctx.enter_context(tc.tile_pool(name="pA", bufs=1, space="PSUM"))
        ppB = ctx.enter_context(tc.tile_pool(name="pB", bufs=1, space="PSUM"))
        ppC = ctx.enter_context(tc.tile_pool(name="pC", bufs=1, space="PSUM"))
        ppM = ctx.enter_context(tc.tile_pool(name="pM", bufs=2, space="PSUM"))
        ppE = ctx.enter_context(tc.tile_pool(name="pE", bufs=1, space="PSUM"))

        wc = cpool.tile([128, WC_COLS], dt.bfloat16, tag="wc")
        nc.sync.dma_start(wc[:], a_wc)
        wdr = cpool.tile([128, 2, 128], dt.float8e4, tag="wdr")
        nc.sync.dma_start(wdr[:], a_wdr)

        def W(name):
            return wc[:, _wslot(name): _wslot(name) + 128]

        kron_g2 = wc[:, EX_G2:EX_G2 + 12]
        ones12 = wc[0:12, EX_ONES:EX_ONES + 12]
        scat = [wc[0:12, EX_SCAT + 128 * e: EX_SCAT + 128 * (e + 1)]
                for e in range(3)]

        off = [0]

        def emit_loads(i):
            v0, v1, v2 = w0s[i], w1s[i], w2s[i]
            vt = v0 + v1 + v2
            ma = pma.tile([128, vt, SC], dt.float8e4, tag="ma")
            nc.sync.dma_start(ma[:], a_ma[:, off[0] * SC:(off[0] + vt) * SC])
            off[0] += vt
            ci = pci.tile([128, 4, SC], dt.bfloat16, tag="ci")
            nc.sync.dma_start(ci[:], a_ci[:, i * 4 * SC:(i + 1) * 4 * SC])
            tnh = ptnh.tile([128, max(w1s), SC], dt.float8e4, tag="tnh")
            return dict(ma=ma, ci=ci, tnh=tnh, v0=v0, v1=v1, v2=v2,
                        cst=ci[:, 0, :], inv0=ci[:, 1, :], inv1=ci[:, 2, :],
                        inv2=ci[:, 3, :])

        def emit_chainB1(st):
            """tier2 DR chain -> bank B (stop for S2 read)."""
            ma, v0, v1, v2 = st["ma"], st["v0"], st["v1"], st["v2"]
            pB = ppB.tile([128, SC], dt.float32, tag="B")
            st["pB"] = pB
            o = v0 + v1
            for p in range(v2 // 2):
                nc.tensor.matmul(pB[:], wdr[:], ma[:, o + 2 * p:o + 2 * p + 2, :],
                                 start=(p == 0), stop=(p == v2 // 2 - 1),
                                 perf_mode=PM.DoubleRow)

        def emit_s2(st):
            """Read S2 out of bank B, mdis = S2 * inv2 (SBUF bf16)."""
            s2c = psml.tile([128, SC], dt.bfloat16, tag="s2c")
            nc.vector.tensor_copy(s2c[:], st["pB"][:])
            mdis = psml.tile([128, SC], dt.bfloat16, tag="mdis")
            nc.vector.tensor_tensor(out=mdis[:], in0=s2c[:], in1=st["inv2"],
                                    op=ALU.mult)
            st["mdis"] = mdis

        def emit_chainA_B2(st):
            ma, v0, v1 = st["ma"], st["v0"], st["v1"]
            pA = ppA.tile([128, SC], dt.float32, tag="A")
            st["pA"] = pA
            for p in range(v0 // 2):
                nc.tensor.matmul(pA[:], wdr[:], ma[:, 2 * p:2 * p + 2, :],
                                 start=(p == 0), stop=(p == v0 // 2 - 1),
                                 perf_mode=PM.DoubleRow)
            pB = st["pB"]
            for p in range(v1 // 2):
                nc.tensor.matmul(pB[:], wdr[:], ma[:, v0 + 2 * p:v0 + 2 * p + 2, :],
                                 start=False, stop=(p == v1 // 2 - 1),
                                 perf_mode=PM.DoubleRow, skip_group_check=True)

        def emit_msg_pair(st, p):
            """Two msg matmuls (bf16 W x fp8 slot) + one wide tanh -> fp8."""
            ma, tnh, v0 = st["ma"], st["tnh"], st["v0"]
            pm = ppM.tile([128, 2, SC], dt.float32, tag="pm")
            j = v0 + 2 * p
            nc.tensor.matmul(pm[:, 0, :], W("W4msg"), ma[:, j:j + 1, :],
                             start=True, stop=True)
            nc.tensor.matmul(pm[:, 1, :], W("W4msg"), ma[:, j + 1:j + 2, :],
                             start=True, stop=True)
            nc.scalar.activation(tnh[:, 2 * p:2 * p + 2, :], pm[:], AF.Tanh)

        def emit_tnh_chain(st):
            tnh, v1 = st["tnh"], st["v1"]
            pC = ppC.tile([128, SC], dt.float32, tag="C")
            st["pC"] = pC
            for p in range(v1 // 2):
                nc.tensor.matmul(pC[:], wdr[:], tnh[:, 2 * p:2 * p + 2, :],
                                 start=(p == 0), stop=(p == v1 // 2 - 1),
                                 perf_mode=PM.DoubleRow)

        def emit_chain_reads(st):
            """mloc/s12c/s0/aggb for iteration st; frees banks A, B, C."""
            pA, pB, pC = st["pA"], st["pB"], st["pC"]
            mloc = psml.tile([128, SC], dt.bfloat16, tag="mloc")
            nc.vector.tensor_tensor(out=mloc[:], in0=pA[:], in1=st["inv0"],
                                    op=ALU.mult)
            s12c = psml.tile([128, SC], dt.bfloat16, tag="s12c")
            nc.vector.tensor_copy(s12c[:], pB[:])
            s0 = psml.tile([128, SC], dt.bfloat16, tag="s0")
            nc.vector.tensor_tensor(out=s0[:], in0=pA[:], in1=s12c[:], op=ALU.add)
            aggb = psml.tile([128, SC], dt.bfloat16, tag="aggb")
            nc.vector.tensor_tensor(out=aggb[:], in0=pC[:], in1=st["inv1"],
                                    op=ALU.mult)
            st.update(mloc=mloc, s0=s0, aggb=aggb)

        def back_units(i, h):
            """Experts/gating/cnf/combine for super-tile i (chain reads done)."""
            cst = h["cst"]
            units = []

            def u_local():
                pl = ppE.tile([128, SC], dt.float32, tag="pe")
                nc.tensor.matmul(pl[:], W("Wl_t"), cst, start=True, stop=False)
                nc.tensor.matmul(pl[:], W("Wl_b"), h["mloc"][:], start=False, stop=True)
                h["locb"] = psml.tile([128, SC], dt.bfloat16, name="locb", tag="locb")
                nc.scalar.activation(h["locb"][:], pl[:], AF.Tanh)
            units.append(u_local)

            def u_func1():
                pu = ppE.tile([128, SC], dt.float32, tag="pe")
                nc.tensor.matmul(pu[:], W("Wu_t"), cst, start=True, stop=False)
                nc.tensor.matmul(pu[:], W("Wu_b"), h["aggb"][:], start=False, stop=True)
                h["tu"] = psml.tile([128, SC], dt.bfloat16, name="tu", tag="tu")
                nc.scalar.activation(h["tu"][:], pu[:], AF.Tanh)
                h["tagg"] = psml.tile([128, SC], dt.bfloat16, name="tagg", tag="tagg")
                nc.scalar.activation(h["tagg"][:], h["aggb"][:], AF.Tanh)
            units.append(u_func1)

            def u_func2():
                d2 = psml.tile([128, SC], dt.bfloat16, tag="d2")
                nc.vector.tensor_tensor(out=d2[:], in0=h["tagg"][:], in1=cst,
                                        op=ALU.subtract)
                e1 = psml.tile([128, SC], dt.bfloat16, tag="e1")
                nc.vector.scalar_tensor_tensor(out=e1[:], in0=h["tu"][:], scalar=0.5,
                                               in1=d2[:], op0=ALU.mult, op1=ALU.mult)
                e2 = psml.tile([128, SC], dt.bfloat16, tag="e2")
                nc.vector.scalar_tensor_tensor(out=e2[:], in0=d2[:], scalar=0.5,
                                               in1=cst, op0=ALU.mult, op1=ALU.add)
                h["funcb"] = psml.tile([128, SC], dt.bfloat16, name="funcb", tag="funcb")
                nc.vector.tensor_tensor(out=h["funcb"][:], in0=e1[:], in1=e2[:],
                                        op=ALU.add)
            units.append(u_func2)

            def u_gate1():
                pg = ppE.tile([128, SC], dt.float32, tag="pe")
                nc.tensor.matmul(pg[:], W("Wg1_t"), cst, start=True, stop=False)
                nc.tensor.matmul(pg[:], W("Wg1_b"), h["s0"][:], start=False, stop=True)
                h["hb"] = psml.tile([128, SC], dt.bfloat16, name="hb", tag="hb")
                nc.vector.tensor_scalar(out=h["hb"][:], in0=pg[:], scalar1=0.0,
                                        scalar2=None, op0=ALU.max)
            units.append(u_gate1)

            def u_cnf0():
                pcnf = ppE.tile([128, SC], dt.float32, tag="pe")
                nc.tensor.matmul(pcnf[:], W("Wc_t"), cst, start=True, stop=False)
                nc.tensor.matmul(pcnf[:], W("Wc_b"), h["mdis"][:], start=False, stop=True)
                h["pcnf"] = pcnf
                h["t0"] = psml.tile([128, SC], dt.bfloat16, name="t0", tag="t0")
                nc.scalar.activation(h["t0"][:], pcnf[:], AF.Tanh)
                nc.tensor.matmul(pcnf[:], W("Wc_td"), h["t0"][:], start=False,
                                 stop=True, skip_group_check=True)
            units.append(u_cnf0)

            def u_cnf1():
                pcnf = h["pcnf"]
                h["t1"] = psml.tile([128, SC], dt.bfloat16, name="t1", tag="t1")
                nc.scalar.activation(h["t1"][:], pcnf[:], AF.Tanh)
                nc.tensor.matmul(pcnf[:], W("Wc_td"), h["t1"][:], start=False,
                                 stop=True, skip_group_check=True)
                h["t01"] = psml.tile([128, SC], dt.bfloat16, name="t01", tag="t01")
                nc.vector.tensor_tensor(out=h["t01"][:], in0=h["t0"][:],
                                        in1=h["t1"][:], op=ALU.add)
            units.append(u_cnf1)

            def u_cnf2():
                pcnf = h["pcnf"]
                t2 = psml.tile([128, SC], dt.bfloat16, tag="t2")
                nc.scalar.activation(t2[:], pcnf[:], AF.Tanh)
                tall = psml.tile([128, SC], dt.bfloat16, tag="tall")
                nc.vector.tensor_tensor(out=tall[:], in0=h["t01"][:], in1=t2[:],
                                        op=ALU.add)
                h["dist"] = psml.tile([128, SC], dt.bfloat16, name="dist", tag="dist")
                nc.vector.scalar_tensor_tensor(out=h["dist"][:], in0=tall[:],
                                               scalar=DT_STEP, in1=cst,
                                               op0=ALU.mult, op1=ALU.add)
            units.append(u_cnf2)

            def u_gate2():
                pl2 = ppE.tile([128, SC], dt.float32, tag="pe")
                nc.tensor.matmul(pl2[0:12, :], kron_g2, h["hb"][:], start=True,
                                 stop=True)
                h["eg"] = psml.tile([12, SC], dt.bfloat16, name="eg", tag="eg")
                nc.scalar.activation(h["eg"][:], pl2[0:12, :], AF.Exp)
                ps = ppE.tile([128, SC], dt.float32, tag="pe")
                nc.tensor.matmul(ps[0:12, :], ones12, h["eg"][:], start=True,
                                 stop=True)
                rec = psml.tile([12, SC], dt.float32, tag="rec")
                nc.vector.reciprocal_approx_fast(out=rec[:], in_=ps[0:12, :])
                h["egn"] = psml.tile([12, SC], dt.bfloat16, name="egn", tag="egn")
                nc.vector.tensor_tensor(out=h["egn"][:], in0=h["eg"][:],
                                        in1=rec[:], op=ALU.mult)
            units.append(u_gate2)

            def u_comb():
                exps = [h["locb"], h["funcb"], h["dist"]]
                aes = []
                for e in range(3):
                    pge = ppM.tile([128, 2, SC], dt.float32, tag="pm")
                    nc.tensor.matmul(pge[:, 0, :], scat[e], h["egn"][:],
                                     start=True, stop=True)
                    ae = psml.tile([128, SC], dt.bfloat16, tag=f"ae{e}")
                    nc.vector.tensor_tensor(out=ae[:], in0=pge[:, 0, :],
                                            in1=exps[e][:], op=ALU.mult)
                    aes.append(ae)
                a12 = psml.tile([128, SC], dt.bfloat16, tag="a12")
                nc.vector.tensor_tensor(out=a12[:], in0=aes[0][:], in1=aes[1][:],
                                        op=ALU.add)
                outb = pout.tile([128, SC], dt.bfloat16, tag="outb")
                nc.vector.tensor_tensor(out=outb[:], in0=a12[:], in1=aes[2][:],
                                        op=ALU.add)
                nc.sync.dma_start(a_out[:, i * SC:(i + 1) * SC], outb[:])
            units.append(u_comb)

            return units

        # --- main pipeline ---
        prev = None
        for i in range(NT):
            st = emit_loads(i)
            if prev is not None:
                emit_chain_reads(prev)          # frees A, B, C of i-1
            emit_chainB1(st)
            emit_s2(st)
            emit_chainA_B2(st)
            # weave msg pairs with back(i-1) units
            units = back_units(i - 1, prev) if prev is not None else []
            npairs = st["v1"] // 2
            n_u = len(units)
            k = 0
            for p in range(npairs):
                emit_msg_pair(st, p)
                while k < n_u and (k + 1) * npairs <= (p + 1) * n_u:
                    units[k]()
                    k += 1
            while k < n_u:
                units[k]()
                k += 1
            emit_tnh_chain(st)
            prev = st
        emit_chain_reads(prev)
        for u in back_units(NT - 1, prev):
            u()


# ---------------------------------------------------------------------------
# host staging
# ---------------------------------------------------------------------------

def _to_blockT(arr_bsd):
    """[n, d] (d == 32, n multiple of 128) -> blockT [128, n//128*32]."""
    n, d = arr_bsd.shape
    a = arr_bsd.reshape(n // 128, 4, 32, d)          # [t, g, c, d]
    a = a.transpose(1, 3, 0, 2)                      # [g, d, t, c]
    return np.ascontiguousarray(a.reshape(128, n // 4))


def _nb_blockT(nb_sel):
    """[n, w, 32] premasked sorted neighbors -> [128, w*n//4] (j outermost)."""
    n, w, d = nb_sel.shape
    a = nb_sel.reshape(n // 128, 4, 32, w, d)        # [t, g, c, j, d]
    a = a.transpose(1, 4, 3, 0, 2)                   # [g, d, j, t, c]
    return np.ascontiguousarray(a.reshape(128, w * n // 4))


def _from_blockT(arr):
    """inverse of _to_blockT per ST block: [128, NT*SC] -> [BS, 32]."""
    a = arr.reshape(4, 32, NT * TPS, 32)             # [g, d, t, c]
    a = a.transpose(2, 0, 3, 1)                      # [t, g, c, d]
    return np.ascontiguousarray(a.reshape(BS, 32))


def _fp8_error_feedback(x):
    """Quantize [n, w, d] to fp8e4 with per-cell carry along slot axis so
    the slot-sum is preserved to ~1 quantization step."""
    n, w, d = x.shape
    q = np.empty((n, w, d), f8e4)
    carry = np.zeros((n, d), np.float32)
    for j in range(w):
        t = x[:, j, :] + carry
        qj = t.astype(f8e4)
        q[:, j, :] = qj
        carry = t - qj.astype(np.float32)
    return q


def stage_weights(inputs):
    f32 = np.float32
    for b in ("b_local", "b_msg", "b_upd", "b_cnf", "b_g1", "b_g2"):
        if np.any(np.asarray(inputs[b], f32) != 0.0):
            raise NotImplementedError(f"kernel requires {b} == 0")

    W_local = np.asarray(inputs["W_local"], f32)
    W_msg = np.asarray(inputs["W_msg"], f32)
    W_upd = np.asarray(inputs["W_upd"], f32)
    W_cnf = np.asarray(inputs["W_cnf"], f32)
    W_g1 = np.asarray(inputs["W_g1"], f32)
    W_g2 = np.asarray(inputs["W_g2"], f32)

    eye4 = np.eye(4, dtype=f32)

    def kron4(w):
        return np.kron(eye4, w)

    wparts = {
        "W4msg": kron4(W_msg),
        "Wl_t": kron4(W_local[:D]), "Wl_b": kron4(W_local[D:]),
        "Wu_t": kron4(0.5 * W_upd[:D]), "Wu_b": kron4(0.5 * W_upd[D:]),
        "Wc_t": kron4(W_cnf[:D]), "Wc_b": kron4(W_cnf[D:]),
        "Wg1_t": kron4(W_g1[:D]), "Wg1_b": kron4(W_g1[D:] / K),
        "Wc_td": kron4(W_cnf[:D] * DT_STEP),
    }
    wc = np.zeros((128, WC_COLS), f32)
    for name in _WSLOTS:
        wc[:, _wslot(name):_wslot(name) + 128] = wparts[name]
    for g in range(4):
        wc[32 * g:32 * (g + 1), EX_G2 + 3 * g:EX_G2 + 3 * (g + 1)] = W_g2
    # ones12: [12, 12] contraction rows (g,e') -> out (g,e): 1 if same g
    for g in range(4):
        for e1 in range(3):
            for e2 in range(3):
                wc[3 * g + e1, EX_ONES + 3 * g + e2] = 1.0
    for e in range(3):
        for g in range(4):
            wc[3 * g + e, EX_SCAT + 128 * e + 32 * g:
               EX_SCAT + 128 * e + 32 * (g + 1)] = 1.0
    wc = wc.astype(bf16)

    eye = np.eye(128, dtype=f32)
    wdr = np.stack([eye, eye], axis=1).astype(f8e4)  # [128, 2, 128]
    return wc, wdr.reshape(128, 2 * 128)


def stage_inputs(inputs):
    """Returns (in_maps, widths, perm)."""
    f32 = np.float32
    cs = np.asarray(inputs["current_state"], f32)
    nb = np.asarray(inputs["neighbor_states"], f32)
    tiers = np.asarray(inputs["tier_ids"], np.int32)

    cnt = np.stack([(tiers == t).sum(-1) for t in range(3)], axis=1)  # [B,3]
    snake_c0 = np.where(cnt[:, 1] % 2 == 0, cnt[:, 0], -cnt[:, 0])
    perm = np.lexsort((snake_c0, cnt[:, 1]))         # rank -> cell

    # per-ST band widths (shared by all cores), padded to even, min 2
    widths = []
    for t in range(3):
        ct = cnt[perm, t].reshape(NT, ST * N_CORES)
        w = np.maximum(2, ct.max(axis=1))
        w = w + (w % 2)
        widths.append(tuple(int(x) for x in w))
    widths = tuple(widths)

    inv = (1.0 / np.maximum(cnt, 1.0)).astype(f32)   # [B, 3]
    wc, wdr = stage_weights(inputs)

    wmax = [max(w) for w in widths]
    orders = []
    for t in range(3):
        order = np.argsort(tiers != t, axis=1, kind="stable")[:, :wmax[t]]
        orders.append(order)

    in_maps = []
    for c in range(N_CORES):
        cells = perm[c::N_CORES]                     # [BS] cell ids
        mall = []
        for i in range(NT):
            cell_i = cells[i * ST:(i + 1) * ST]
            for t in range(3):
                w = widths[t][i]
                od = orders[t][cell_i, :w]
                sel = np.take_along_axis(nb[cell_i], od[:, :, None], axis=1)
                msk = np.take_along_axis(tiers[cell_i], od, axis=1) == t
                x = (sel * msk[:, :, None]).astype(f32)
                if t == 1:
                    q = x.astype(f8e4)               # per-slot accuracy
                else:
                    q = _fp8_error_feedback(x)       # sum accuracy
                # blockT with j outermost, fp8 via f32 view for _nb_blockT
                mall.append(_nb_blockT(q.astype(f32)).astype(f8e4))
        ci = np.empty((128, NT * 4 * SC), f32)
        for i in range(NT):
            cell_i = cells[i * ST:(i + 1) * ST]
            ci[:, (4 * i) * SC:(4 * i + 1) * SC] = _to_blockT(cs[cell_i])
            for t in range(3):
                ci[:, (4 * i + 1 + t) * SC:(4 * i + 2 + t) * SC] = \
                    _to_blockT(np.repeat(inv[cell_i, t:t + 1], D, axis=1))
        in_maps.append({
            "ma": np.concatenate(mall, axis=1),
            "ci": ci.astype(bf16),
            "wc": wc, "wdr": wdr,
        })
    return in_maps, widths, perm


_PROGRAM_CACHE = {}


def kernel(**inputs):
    from concourse.bass_utils import run_bass_kernel_spmd

    in_maps, widths, perm = stage_inputs(inputs)
    if widths not in _PROGRAM_CACHE:
        _PROGRAM_CACHE[widths] = build_program(widths)
    nc = _PROGRAM_CACHE[widths]

    res = run_bass_kernel_spmd(nc, in_maps, core_ids=list(range(N_CORES)))
    out = np.empty((B, D), np.float32)
    for c in range(N_CORES):
        out[perm[c::N_CORES]] = _from_blockT(
            np.asarray(res.results[c]["out"], np.float32))
    return out


# revision 13
# speedup vs baseline: 1.2987x; 1.0267x over previous
"""Trainium2 Bass kernel for nn_MoEConnectionProcessor (v3: fp8 DoubleRow chains).

Strategy (delta over v2)
------------------------
Data-parallel over 8 cores; per core 16 super-tiles (ST) of 2048 cells in
blockT layout: SBUF partition = (g, d) (cell-subgroup x feature), free
axis = (t, c) = 512 cols per ST.

v3 structural changes:
  * The three premasked neighbor copies are staged as ONE concatenated
    fp8e4m3 tensor [tier0 | tier1 | tier2] with per-band widths padded to
    EVEN.  m0/m2 use host-side error-feedback quantization (per-cell carry
    across slots) so the tier SUMS keep near-bf16 accuracy.
  * All slot sums run on the PE as DoubleRow fp8 accumulation chains
    (2 slots per matmul) with a single shared stationary (I128 pair):
      bank B: tier2 -> (read S2) -> continue tier1 -> S12
      bank A: tier0 -> S0
      bank C: tanh(msg) slots (fp8 pairs written by ACT) -> T1
    s0 = S0 + S12, mdis = S2*inv2, mloc = S0*inv0, agg = T1*inv1.
    This removes all DVE fold trees.
  * msg matmuls: bf16 kron4(W_msg) stationary x fp8 slots (mixed dtypes),
    two slots into a 2-bank PSUM tile, ONE wide ACT tanh per pair writing
    fp8 pairs for the DR tanh-sum chain.
  * All biases are zero by spec -> dropped (asserted at staging); the
    sigmoid 1/2 scale is folded into W_upd so local/upd share plain tanh.
  * Gates are normalized (exp * 1/sum) before the scatter broadcast,
    removing the reciprocal-broadcast matmul and final divide.
  * PE weight churn minimized: chains+tanh-sum share one DR stationary.

PSUM banks: A(1) B(1) C(1) + msg pairs 2x[128,2SC](4) + expert(1) = 8.
"""

import numpy as np
import ml_dtypes
from contextlib import ExitStack

import concourse.bass as bass
import concourse.bacc as bacc
import concourse.tile as tile
import concourse.mybir as mybir

B, K, D, NH = 262144, 26, 32, 32
N_CORES = 8
BS = B // N_CORES          # 32768 cells per core
ST = 2048                  # cells per super-tile
NT = BS // ST              # 16 super-tiles per core
TPS = ST // 128            # 16 tiles of 128 cells per super-tile
SC = TPS * 32              # 512 free columns per super-tile (t, c)
N_STEPS = 3
DT_STEP = 1.0 / N_STEPS

dt = mybir.dt
bf16 = ml_dtypes.bfloat16
f8e4 = ml_dtypes.float8_e4m3
AF = mybir.ActivationFunctionType
ALU = mybir.AluOpType
PM = mybir.MatmulPerfMode

# bf16 stationary slots in wc: [128, n*128 + 12 + 12 + 3*128]
_WSLOTS = ["W4msg", "Wl_t", "Wl_b", "Wu_t", "Wu_b", "Wc_t", "Wc_b",
           "Wg1_t", "Wg1_b", "Wc_td"]
EX_G2 = 128 * len(_WSLOTS)          # kron(I4, W_g2): [128, 12]
EX_ONES = EX_G2 + 12                # ones12: [12, 12] group-sum bcast
EX_SCAT = EX_ONES + 12              # gate scatter e=0..2: [12, 128] each
WC_COLS = EX_SCAT + 3 * 128


def _wslot(name):
    return 128 * _WSLOTS.index(name)


def build_program(widths):
    nc = bacc.Bacc("TRN2", target_bir_lowering=False, debug=False,
                   num_devices=N_CORES)

    totc = sum(v0 + v1 + v2 for v0, v1, v2 in zip(*widths))
    a_ma = nc.dram_tensor("ma", [128, totc * SC], dt.float8e4,
                          kind="ExternalInput").ap()
    a_ci = nc.dram_tensor("ci", [128, NT * 4 * SC], dt.bfloat16,
                          kind="ExternalInput").ap()
    a_wc = nc.dram_tensor("wc", [128, WC_COLS], dt.bfloat16,
                          kind="ExternalInput").ap()
    a_wdr = nc.dram_tensor("wdr", [128, 2 * 128], dt.float8e4,
                           kind="ExternalInput").ap()
    a_out = nc.dram_tensor("out", [128, NT * SC], dt.bfloat16,
                           kind="ExternalOutput").ap()

    with tile.TileContext(nc) as tc:
        _body(tc, a_ma, a_ci, a_wc, a_wdr, a_out, widths)
    nc.compile()
    return nc


def _body(tc, a_ma, a_ci, a_wc, a_wdr, a_out, widths):
    nc = tc.nc
    w0s, w1s, w2s = widths

    with ExitStack() as ctx:
        cpool = ctx.enter_context(tc.tile_pool(name="const", bufs=1))
        pma = ctx.enter_context(tc.tile_pool(name="ma", bufs=2))
        pci = ctx.enter_context(tc.tile_pool(name="ci", bufs=2))
        ptnh = ctx.enter_context(tc.tile_pool(name="tnh", bufs=2))
        psml = ctx.enter_context(tc.tile_pool(name="sml", bufs=2))
        pout = ctx.enter_context(tc.tile_pool(name="out", bufs=2))
        # PSUM: chain pool {A, B, T1} rotating over 3 banks + msg 2x2 +
        # expert 1 = 8 banks.  Chain-freeing DVE reads are emitted before
        # the weave so the next ST's chains never wait (keeps PE warm).
        ppCH = ctx.enter_context(tc.tile_pool(name="pCH", bufs=3, space="PSUM"))
        ppM = ctx.enter_context(tc.tile_pool(name="pM", bufs=2, space="PSUM"))
        ppE = ctx.enter_context(tc.tile_pool(name="pE", bufs=1, space="PSUM"))

        wc = cpool.tile([128, WC_COLS], dt.bfloat16, tag="wc")
        nc.sync.dma_start(wc[:], a_wc)
        wdr = cpool.tile([128, 2, 128], dt.float8e4, tag="wdr")
        nc.sync.dma_start(wdr[:], a_wdr)

        def W(name):
            return wc[:, _wslot(name): _wslot(name) + 128]

        kron_g2 = wc[:, EX_G2:EX_G2 + 12]
        ones12 = wc[0:12, EX_ONES:EX_ONES + 12]
        scat = [wc[0:12, EX_SCAT + 128 * e: EX_SCAT + 128 * (e + 1)]
                for e in range(3)]

        off = [0]

        def emit_loads(i):
            v0, v1, v2 = w0s[i], w1s[i], w2s[i]
            vt = v0 + v1 + v2
            ma = pma.tile([128, vt, SC], dt.float8e4, tag="ma")
            nc.sync.dma_start(ma[:], a_ma[:, off[0] * SC:(off[0] + vt) * SC])
            off[0] += vt
            ci = pci.tile([128, 4, SC], dt.bfloat16, tag="ci")
            nc.sync.dma_start(ci[:], a_ci[:, i * 4 * SC:(i + 1) * 4 * SC])
            tnh = ptnh.tile([128, max(w1s), SC], dt.float8e4, tag="tnh")
            return dict(ma=ma, ci=ci, tnh=tnh, v0=v0, v1=v1, v2=v2,
                        cst=ci[:, 0, :], inv0=ci[:, 1, :], inv1=ci[:, 2, :],
                        inv2=ci[:, 3, :])

        def emit_chainB1(st):
            """tier2 DR chain -> bank B (stop for S2 read)."""
            ma, v0, v1, v2 = st["ma"], st["v0"], st["v1"], st["v2"]
            pA = ppCH.tile([128, SC], dt.float32, name="pA", tag="ch")
            pB = ppCH.tile([128, SC], dt.float32, name="pB", tag="ch")
            st["pA"], st["pB"] = pA, pB
            o = v0 + v1
            for p in range(v2 // 2):
                nc.tensor.matmul(pB[:], wdr[:], ma[:, o + 2 * p:o + 2 * p + 2, :],
                                 start=(p == 0), stop=(p == v2 // 2 - 1),
                                 perf_mode=PM.DoubleRow)

        def emit_s2(st):
            """Read S2 out of bank B, mdis = S2 * inv2 (SBUF bf16)."""
            s2c = psml.tile([128, SC], dt.bfloat16, tag="s2c")
            nc.vector.tensor_copy(s2c[:], st["pB"][:])
            mdis = psml.tile([128, SC], dt.bfloat16, tag="mdis")
            nc.vector.tensor_tensor(out=mdis[:], in0=s2c[:], in1=st["inv2"],
                                    op=ALU.mult)
            st["mdis"] = mdis

        def emit_chainA_B2(st):
            ma, v0, v1 = st["ma"], st["v0"], st["v1"]
            pA = st["pA"]
            for p in range(v0 // 2):
                nc.tensor.matmul(pA[:], wdr[:], ma[:, 2 * p:2 * p + 2, :],
                                 start=(p == 0), stop=(p == v0 // 2 - 1),
                                 perf_mode=PM.DoubleRow)
            pB = st["pB"]
            for p in range(v1 // 2):
                nc.tensor.matmul(pB[:], wdr[:], ma[:, v0 + 2 * p:v0 + 2 * p + 2, :],
                                 start=False, stop=(p == v1 // 2 - 1),
                                 perf_mode=PM.DoubleRow, skip_group_check=True)

        def emit_msg_pair(st, p):
            """Two msg matmuls (bf16 W x fp8 slot) + one wide tanh -> fp8."""
            ma, tnh, v0 = st["ma"], st["tnh"], st["v0"]
            pm = ppM.tile([128, 2, SC], dt.float32, tag="pm")
            j = v0 + 2 * p
            nc.tensor.matmul(pm[:, 0, :], W("W4msg"), ma[:, j:j + 1, :],
                             start=True, stop=True)
            nc.tensor.matmul(pm[:, 1, :], W("W4msg"), ma[:, j + 1:j + 2, :],
                             start=True, stop=True)
            nc.scalar.activation(tnh[:, 2 * p:2 * p + 2, :], pm[:], AF.Tanh)

        def emit_AB_reads(st):
            """mloc/s12c/s0 right after the chains; frees banks A and B."""
            pA, pB = st["pA"], st["pB"]
            mloc = psml.tile([128, SC], dt.bfloat16, tag="mloc")
            nc.vector.tensor_tensor(out=mloc[:], in0=pA[:], in1=st["inv0"],
                                    op=ALU.mult)
            s12c = psml.tile([128, SC], dt.bfloat16, tag="s12c")
            nc.vector.tensor_copy(s12c[:], pB[:])
            s0 = psml.tile([128, SC], dt.bfloat16, tag="s0")
            nc.vector.tensor_tensor(out=s0[:], in0=pA[:], in1=s12c[:], op=ALU.add)
            st.update(mloc=mloc, s0=s0)

        def emit_tnh_chain(st):
            tnh, v1 = st["tnh"], st["v1"]
            pC = ppCH.tile([128, SC], dt.float32, name="pC", tag="ch")
            st["pC"] = pC
            for p in range(v1 // 2):
                nc.tensor.matmul(pC[:], wdr[:], tnh[:, 2 * p:2 * p + 2, :],
                                 start=(p == 0), stop=(p == v1 // 2 - 1),
                                 perf_mode=PM.DoubleRow)
            aggb = psml.tile([128, SC], dt.bfloat16, tag="aggb")
            nc.vector.tensor_tensor(out=aggb[:], in0=pC[:], in1=st["inv1"],
                                    op=ALU.mult)
            st["aggb"] = aggb

        def back_units(i, h):
            """Experts/gating/cnf/combine for super-tile i (chain reads done)."""
            cst = h["cst"]
            units = []

            def u_local():
                pl = ppE.tile([128, SC], dt.float32, tag="pe")
                nc.tensor.matmul(pl[:], W("Wl_t"), cst, start=True, stop=False)
                nc.tensor.matmul(pl[:], W("Wl_b"), h["mloc"][:], start=False, stop=True)
                h["locb"] = psml.tile([128, SC], dt.bfloat16, name="locb", tag="locb")
                nc.scalar.activation(h["locb"][:], pl[:], AF.Tanh)
            units.append(u_local)

            def u_func1():
                pu = ppE.tile([128, SC], dt.float32, tag="pe")
                nc.tensor.matmul(pu[:], W("Wu_t"), cst, start=True, stop=False)
                nc.tensor.matmul(pu[:], W("Wu_b"), h["aggb"][:], start=False, stop=True)
                h["tu"] = psml.tile([128, SC], dt.bfloat16, name="tu", tag="tu")
                nc.scalar.activation(h["tu"][:], pu[:], AF.Tanh)
                h["tagg"] = psml.tile([128, SC], dt.bfloat16, name="tagg", tag="tagg")
                nc.scalar.activation(h["tagg"][:], h["aggb"][:], AF.Tanh)
            units.append(u_func1)

            def u_func2():
                d2 = psml.tile([128, SC], dt.bfloat16, tag="d2")
                nc.vector.tensor_tensor(out=d2[:], in0=h["tagg"][:], in1=cst,
                                        op=ALU.subtract)
                e1 = psml.tile([128, SC], dt.bfloat16, tag="e1")
                nc.vector.scalar_tensor_tensor(out=e1[:], in0=h["tu"][:], scalar=0.5,
                                               in1=d2[:], op0=ALU.mult, op1=ALU.mult)
                e2 = psml.tile([128, SC], dt.bfloat16, tag="e2")
                nc.vector.scalar_tensor_tensor(out=e2[:], in0=d2[:], scalar=0.5,
                                               in1=cst, op0=ALU.mult, op1=ALU.add)
                h["funcb"] = psml.tile([128, SC], dt.bfloat16, name="funcb", tag="funcb")
                nc.vector.tensor_tensor(out=h["funcb"][:], in0=e1[:], in1=e2[:],
                                        op=ALU.add)
            units.append(u_func2)

            def u_gate1():
                pg = ppE.tile([128, SC], dt.float32, tag="pe")
                nc.tensor.matmul(pg[:], W("Wg1_t"), cst, start=True, stop=False)
                nc.tensor.matmul(pg[:], W("Wg1_b"), h["s0"][:], start=False, stop=True)
                h["hb"] = psml.tile([128, SC], dt.bfloat16, name="hb", tag="hb")
                nc.vector.tensor_scalar(out=h["hb"][:], in0=pg[:], scalar1=0.0,
                                        scalar2=None, op0=ALU.max)
            units.append(u_gate1)

            def u_cnf0():
                pcnf = ppE.tile([128, SC], dt.float32, tag="pe")
                nc.tensor.matmul(pcnf[:], W("Wc_t"), cst, start=True, stop=False)
                nc.tensor.matmul(pcnf[:], W("Wc_b"), h["mdis"][:], start=False, stop=True)
                h["pcnf"] = pcnf
                h["t0"] = psml.tile([128, SC], dt.bfloat16, name="t0", tag="t0")
                nc.scalar.activation(h["t0"][:], pcnf[:], AF.Tanh)
                nc.tensor.matmul(pcnf[:], W("Wc_td"), h["t0"][:], start=False,
                                 stop=True, skip_group_check=True)
            units.append(u_cnf0)

            def u_cnf1():
                pcnf = h["pcnf"]
                h["t1"] = psml.tile([128, SC], dt.bfloat16, name="t1", tag="t1")
                nc.scalar.activation(h["t1"][:], pcnf[:], AF.Tanh)
                nc.tensor.matmul(pcnf[:], W("Wc_td"), h["t1"][:], start=False,
                                 stop=True, skip_group_check=True)
                h["t01"] = psml.tile([128, SC], dt.bfloat16, name="t01", tag="t01")
                nc.vector.tensor_tensor(out=h["t01"][:], in0=h["t0"][:],
                                        in1=h["t1"][:], op=ALU.add)
            units.append(u_cnf1)

            def u_cnf2():
                pcnf = h["pcnf"]
                t2 = psml.tile([128, SC], dt.bfloat16, tag="t2")
                nc.scalar.activation(t2[:], pcnf[:], AF.Tanh)
                tall = psml.tile([128, SC], dt.bfloat16, tag="tall")
                nc.vector.tensor_tensor(out=tall[:], in0=h["t01"][:], in1=t2[:],
                                        op=ALU.add)
                h["dist"] = psml.tile([128, SC], dt.bfloat16, name="dist", tag="dist")
                nc.vector.scalar_tensor_tensor(out=h["dist"][:], in0=tall[:],
                                               scalar=DT_STEP, in1=cst,
                                               op0=ALU.mult, op1=ALU.add)
            units.append(u_cnf2)

            def u_gate2():
                pl2 = ppE.tile([128, SC], dt.float32, tag="pe")
                nc.tensor.matmul(pl2[0:12, :], kron_g2, h["hb"][:], start=True,
                                 stop=True)
                h["eg"] = psml.tile([12, SC], dt.bfloat16, name="eg", tag="eg")
                nc.scalar.activation(h["eg"][:], pl2[0:12, :], AF.Exp)
                ps = ppE.tile([128, SC], dt.float32, tag="pe")
                nc.tensor.matmul(ps[0:12, :], ones12, h["eg"][:], start=True,
                                 stop=True)
                rec = psml.tile([12, SC], dt.float32, tag="rec")
                nc.vector.reciprocal_approx_fast(out=rec[:], in_=ps[0:12, :])
                h["egn"] = psml.tile([12, SC], dt.bfloat16, name="egn", tag="egn")
                nc.vector.tensor_tensor(out=h["egn"][:], in0=h["eg"][:],
                                        in1=rec[:], op=ALU.mult)
            units.append(u_gate2)

            def u_comb():
                exps = [h["locb"], h["funcb"], h["dist"]]
                aes = []
                for e in range(3):
                    pge = ppM.tile([128, 2, SC], dt.float32, tag="pm")
                    nc.tensor.matmul(pge[:, 0, :], scat[e], h["egn"][:],
                                     start=True, stop=True)
                    ae = psml.tile([128, SC], dt.bfloat16, tag=f"ae{e}")
                    nc.vector.tensor_tensor(out=ae[:], in0=pge[:, 0, :],
                                            in1=exps[e][:], op=ALU.mult)
                    aes.append(ae)
                a12 = psml.tile([128, SC], dt.bfloat16, tag="a12")
                nc.vector.tensor_tensor(out=a12[:], in0=aes[0][:], in1=aes[1][:],
                                        op=ALU.add)
                outb = pout.tile([128, SC], dt.bfloat16, tag="outb")
                nc.vector.tensor_tensor(out=outb[:], in0=a12[:], in1=aes[2][:],
                                        op=ALU.add)
                nc.sync.dma_start(a_out[:, i * SC:(i + 1) * SC], outb[:])
            units.append(u_comb)

            return units

        # --- main pipeline ---
        prev = None
        for i in range(NT):
            st = emit_loads(i)
            emit_chainB1(st)
            emit_s2(st)
            emit_chainA_B2(st)
            emit_AB_reads(st)
            # weave msg pairs with back(i-1) units
            units = back_units(i - 1, prev) if prev is not None else []
            npairs = st["v1"] // 2
            n_u = len(units)
            k = 0
            for p in range(npairs):
                emit_msg_pair(st, p)
                while k < n_u and (k + 1) * npairs <= (p + 1) * n_u:
                    units[k]()
                    k += 1
            while k < n_u:
                units[k]()
                k += 1
            emit_tnh_chain(st)
            prev = st
        for u in back_units(NT - 1, prev):
            u()


# ---------------------------------------------------------------------------
# host staging
# ---------------------------------------------------------------------------

def _to_blockT(arr_bsd):
    """[n, d] (d == 32, n multiple of 128) -> blockT [128, n//128*32]."""
    n, d = arr_bsd.shape
    a = arr_bsd.reshape(n // 128, 4, 32, d)          # [t, g, c, d]
    a = a.transpose(1, 3, 0, 2)                      # [g, d, t, c]
    return np.ascontiguousarray(a.reshape(128, n // 4))


def _nb_blockT(nb_sel):
    """[n, w, 32] premasked sorted neighbors -> [128, w*n//4] (j outermost)."""
    n, w, d = nb_sel.shape
    a = nb_sel.reshape(n // 128, 4, 32, w, d)        # [t, g, c, j, d]
    a = a.transpose(1, 4, 3, 0, 2)                   # [g, d, j, t, c]
    return np.ascontiguousarray(a.reshape(128, w * n // 4))


def _from_blockT(arr):
    """inverse of _to_blockT per ST block: [128, NT*SC] -> [BS, 32]."""
    a = arr.reshape(4, 32, NT * TPS, 32)             # [g, d, t, c]
    a = a.transpose(2, 0, 3, 1)                      # [t, g, c, d]
    return np.ascontiguousarray(a.reshape(BS, 32))


def _fp8_error_feedback(x):
    """Quantize [n, w, d] to fp8e4 with per-cell carry along slot axis so
    the slot-sum is preserved to ~1 quantization step."""
    n, w, d = x.shape
    q = np.empty((n, w, d), f8e4)
    carry = np.zeros((n, d), np.float32)
    for j in range(w):
        t = x[:, j, :] + carry
        qj = t.astype(f8e4)
        q[:, j, :] = qj
        carry = t - qj.astype(np.float32)
    return q


def stage_weights(inputs):
    f32 = np.float32
    for b in ("b_local", "b_msg", "b_upd", "b_cnf", "b_g1", "b_g2"):
        if np.any(np.asarray(inputs[b], f32) != 0.0):
            raise NotImplementedError(f"kernel requires {b} == 0")

    W_local = np.asarray(inputs["W_local"], f32)
    W_msg = np.asarray(inputs["W_msg"], f32)
    W_upd = np.asarray(inputs["W_upd"], f32)
    W_cnf = np.asarray(inputs["W_cnf"], f32)
    W_g1 = np.asarray(inputs["W_g1"], f32)
    W_g2 = np.asarray(inputs["W_g2"], f32)

    eye4 = np.eye(4, dtype=f32)

    def kron4(w):
        return np.kron(eye4, w)

    wparts = {
        "W4msg": kron4(W_msg),
        "Wl_t": kron4(W_local[:D]), "Wl_b": kron4(W_local[D:]),
        "Wu_t": kron4(0.5 * W_upd[:D]), "Wu_b": kron4(0.5 * W_upd[D:]),
        "Wc_t": kron4(W_cnf[:D]), "Wc_b": kron4(W_cnf[D:]),
        "Wg1_t": kron4(W_g1[:D]), "Wg1_b": kron4(W_g1[D:] / K),
        "Wc_td": kron4(W_cnf[:D] * DT_STEP),
    }
    wc = np.zeros((128, WC_COLS), f32)
    for name in _WSLOTS:
        wc[:, _wslot(name):_wslot(name) + 128] = wparts[name]
    for g in range(4):
        wc[32 * g:32 * (g + 1), EX_G2 + 3 * g:EX_G2 + 3 * (g + 1)] = W_g2
    # ones12: [12, 12] contraction rows (g,e') -> out (g,e): 1 if same g
    for g in range(4):
        for e1 in range(3):
            for e2 in range(3):
                wc[3 * g + e1, EX_ONES + 3 * g + e2] = 1.0
    for e in range(3):
        for g in range(4):
            wc[3 * g + e, EX_SCAT + 128 * e + 32 * g:
               EX_SCAT + 128 * e + 32 * (g + 1)] = 1.0
    wc = wc.astype(bf16)

    eye = np.eye(128, dtype=f32)
    wdr = np.stack([eye, eye], axis=1).astype(f8e4)  # [128, 2, 128]
    return wc, wdr.reshape(128, 2 * 128)


def stage_inputs(inputs):
    """Returns (in_maps, widths, perm)."""
    f32 = np.float32
    cs = np.asarray(inputs["current_state"], f32)
    nb = np.asarray(inputs["neighbor_states"], f32)
    tiers = np.asarray(inputs["tier_ids"], np.int32)

    cnt = np.stack([(tiers == t).sum(-1) for t in range(3)], axis=1)  # [B,3]
    snake_c0 = np.where(cnt[:, 1] % 2 == 0, cnt[:, 0], -cnt[:, 0])
    perm = np.lexsort((snake_c0, cnt[:, 1]))         # rank -> cell

    # per-ST band widths (shared by all cores), padded to even, min 2
    widths = []
    for t in range(3):
        ct = cnt[perm, t].reshape(NT, ST * N_CORES)
        w = np.maximum(2, ct.max(axis=1))
        w = w + (w % 2)
        widths.append(tuple(int(x) for x in w))
    widths = tuple(widths)

    inv = (1.0 / np.maximum(cnt, 1.0)).astype(f32)   # [B, 3]
    wc, wdr = stage_weights(inputs)

    wmax = [max(w) for w in widths]
    orders = []
    for t in range(3):
        order = np.argsort(tiers != t, axis=1, kind="stable")[:, :wmax[t]]
        orders.append(order)

    in_maps = []
    for c in range(N_CORES):
        cells = perm[c::N_CORES]                     # [BS] cell ids
        mall = []
        for i in range(NT):
            cell_i = cells[i * ST:(i + 1) * ST]
            for t in range(3):
                w = widths[t][i]
                od = orders[t][cell_i, :w]
                sel = np.take_along_axis(nb[cell_i], od[:, :, None], axis=1)
                msk = np.take_along_axis(tiers[cell_i], od, axis=1) == t
                x = (sel * msk[:, :, None]).astype(f32)
                if t == 1:
                    q = x.astype(f8e4)               # per-slot accuracy
                else:
                    q = _fp8_error_feedback(x)       # sum accuracy
                # blockT with j outermost, fp8 via f32 view for _nb_blockT
                mall.append(_nb_blockT(q.astype(f32)).astype(f8e4))
        ci = np.empty((128, NT * 4 * SC), f32)
        for i in range(NT):
            cell_i = cells[i * ST:(i + 1) * ST]
            ci[:, (4 * i) * SC:(4 * i + 1) * SC] = _to_blockT(cs[cell_i])
            for t in range(3):
                ci[:, (4 * i + 1 + t) * SC:(4 * i + 2 + t) * SC] = \
                    _to_blockT(np.repeat(inv[cell_i, t:t + 1], D, axis=1))
        in_maps.append({
            "ma": np.concatenate(mall, axis=1),
            "ci": ci.astype(bf16),
            "wc": wc, "wdr": wdr,
        })
    return in_maps, widths, perm


_PROGRAM_CACHE = {}


def kernel(**inputs):
    from concourse.bass_utils import run_bass_kernel_spmd

    in_maps, widths, perm = stage_inputs(inputs)
    if widths not in _PROGRAM_CACHE:
        _PROGRAM_CACHE[widths] = build_program(widths)
    nc = _PROGRAM_CACHE[widths]

    res = run_bass_kernel_spmd(nc, in_maps, core_ids=list(range(N_CORES)))
    out = np.empty((B, D), np.float32)
    for c in range(N_CORES):
        out[perm[c::N_CORES]] = _from_blockT(
            np.asarray(res.results[c]["out"], np.float32))
    return out
